# revision 119
# baseline (speedup 1.0000x reference)
"""Trainium2 Bass kernel for HIVNet GCN message passing (8-core SPMD).

Final design (baseline 826us -> ~510us; lineage: v7 fp8 DoubleRow 638us,
v8 transposed-h 628us, v11 A-resident 554us):
  - Pad N=10000 nodes to 10240 = 80 chunks x 128; core c owns 10 dst-blocks.
  - Aggregation = dense one-hot adjacency on TensorE via fp8e4m3 DoubleRow
    matmuls (both operands fp8, contraction 256/instruction, 2x bf16 rate).
    A (edge multiplicities, exact in fp8) is layer-invariant and fully
    SBUF-RESIDENT (13.1MB, loaded once) - no per-layer restream, no SBUF
    write contention against the chains.
  - Per layer: hws = (h @ W[l])*nrm*32 fp8 on the owned shard (x32 keeps the
    table out of fp8 subnormals; the dst-side norm carries 1/32), AllGathered
    in 4 pieces {2,2,4,2} posted progressively during the BN-apply loop;
    aggregation runs in 5 segments that consume each piece as it lands
    (Shared-output collectives; tab loads ride the gpsimd queue so a
    blocked trigger never stalls anything else).
  - Layer 0 needs NO gather: every core builds the full layer-0 table
    locally from the (tiny) full feature matrix via
    h0@W0 = x@(D@W0) + base@W0, one 10x128x256 matmul per chunk.
  - h lives TRANSPOSED in bf16 (H on partitions): the GEMM consumes h^T
    directly as lhsT; BN apply is ONE fused Scalar-engine op
    relu(t^T*scale + bias) with per-partition a,c + a bf16 DVE residual add.
  - BN stats: channel sums via PSUM-accumulated ones-matmuls, square on
    the Scalar engine + accumulate on GpSimd (one op per engine per block
    keeps every queue ahead of the chains), then an 8-row-replicated
    Shared ALLGATHER (cheaper than an AllReduce: one row per core is read
    back and the cross-core reduction folds for free into the 4 K=8
    column-ize matmuls); t^T transposes fill the collective window and all
    BN math runs on [128,2] columns.
  - Warmup AllGather+AllReduce at t=0 absorb the ~60us comms boot under the
    encoder; input loads are queue-routed by first-use time.
  - Readout: W1 is folded into the pool sums BEFORE the 128KB pool
    AllReduce (split in two, first half posted early), then rcnt/relu and
    the rest of the MLP run redundantly on every core.
"""

import os
import sys

sys.path.insert(0, "/opt/trn_rl_repo")

from contextlib import ExitStack

import numpy as np
import ml_dtypes

from concourse import bass, mybir, bacc, tile, library_config
from concourse.bass_utils import run_bass_kernel_spmd
from concourse.masks import make_identity

NCORE = 8
P = 128
H = 256
L = 4
NF = 9
G = 256
N = 10000
BPC = 10                # dst blocks per core
NPC = BPC * P           # 1280 nodes per core
NPAD = NCORE * NPC      # 10240
NCHUNK = NPAD // P      # 80 src chunks
HB = BPC // 2           # blocks per AllGather half
BN_EPS = 1e-5
TSCALE = 32.0           # fp8 table scale

f32 = mybir.dt.float32
bf16 = mybir.dt.bfloat16
f8 = mybir.dt.float8e4
bfnp = ml_dtypes.bfloat16

FT = mybir.ActivationFunctionType
OP = mybir.AluOpType
DR = mybir.MatmulPerfMode.DoubleRow

_compiled = {}

NSEG = 5                # aggregation segments per layer (2 blocks each)
BPS = BPC // NSEG       # blocks per segment
CPS = NCORE * BPS       # chunks per segment (16)
# AllGather piece geometry (layers 1..3): small leading pieces so segment 0
# can start early, the 4-block piece in the middle (posted as soon as block
# 7 is ready) so the trailing fifths land with slack.
PW = [2, 2, 4, 2]       # blocks per gather piece
BOFF = [0, 2, 4, 8]     # first block of each piece

# chunk consumption order: fifth-major (blocks {2s,2s+1} of every core form
# table segment s); within a segment, core-major ascending = the gathered
# tab layout.
CHUNK_ORDER = [g for s in range(NSEG) for g in range(NCHUNK)
               if g % BPC in (2 * s, 2 * s + 1)]


# --------------------------------------------------------------------------
# host-side structural preprocessing
# --------------------------------------------------------------------------

def _preprocess(x, edge_index, batch_ids, emb, W, gamma, beta,
                mlp_W1, mlp_b1, mlp_W2, mlp_b2, mlp_W3, mlp_b3):
    src = np.asarray(edge_index[0], np.int64)
    dst = np.asarray(edge_index[1], np.int64)
    # self loops for every real node (weight nrm[d]^2 folds in)
    src_all = np.concatenate([src, np.arange(N, dtype=np.int64)])
    dst_all = np.concatenate([dst, np.arange(N, dtype=np.int64)])
    order = np.argsort(dst_all, kind="stable")
    s_sorted = src_all[order]
    d_sorted = dst_all[order]

    deg = np.bincount(dst_all, minlength=NPAD).astype(np.float64)  # incl self

    nblk = NCORE * BPC
    starts = np.searchsorted(d_sorted, np.arange(nblk) * P)
    ends = np.searchsorted(d_sorted, (np.arange(nblk) + 1) * P)

    # dense adjacency per dst block, chunk-major in CHUNK_ORDER. A is
    # layer-invariant and lives fully resident in SBUF on-device (13.1MB),
    # so own chunks stay in-place (no local/streamed split).
    A_blocks = {}
    for g in range(nblk):
        c, nb = divmod(g, BPC)
        e_s = s_sorted[starts[g]:ends[g]]
        e_d = d_sorted[starts[g]:ends[g]] - g * P
        A = np.zeros((NPAD, P), np.float32)
        np.add.at(A, (e_s, e_d), 1.0)
        A = A.reshape(NCHUNK, P, P)
        A = A[CHUNK_ORDER]                                # reorder chunks
        # fp8 e4m3: edge multiplicities (<= 3 incl. self loop) are exact,
        # and fp8 x fp8 DoubleRow matmul runs at 2x bf16 throughput.
        A_blocks[(c, nb)] = np.ascontiguousarray(
            A.transpose(1, 0, 2).reshape(P, NCHUNK * P)
        ).astype(ml_dtypes.float8_e4m3)

    # graph pool one-hot [node, graph] (bf16: values 0/1 exact)
    bids = np.asarray(batch_ids, np.int64)
    psel_full = np.zeros((NPAD, G), np.float32)
    psel_full[np.arange(N), bids] = 1.0
    cnt = np.bincount(bids, minlength=G).astype(np.float64)
    rcnt = (1.0 / np.maximum(cnt, 1.0)).astype(np.float32)[None, :]

    x_np = np.zeros((NPAD, NF), np.float32)
    x_np[:N] = np.asarray(x, np.float64)

    Wf = np.asarray(W, np.float32)
    embf0 = np.asarray(emb, np.float32)

    # layer-0 table is built LOCALLY on every core (no gather): the full
    # feature matrix is tiny, and h0@W0 = x@(D@W0) + base@W0 collapses the
    # encoder+GEMM into one 10x128x256 matmul per chunk. All in tab-position
    # (fifth-major CHUNK_ORDER) node order.
    Df = embf0[:, 1, :] - embf0[:, 0, :]            # [9, H]
    basef = embf0[:, 0, :].sum(axis=0)              # [H]
    DW0p = np.concatenate([Df @ Wf[0], (basef @ Wf[0])[None, :]], axis=0)
    perm_nodes = np.concatenate(
        [np.arange(g * P, (g + 1) * P) for g in CHUNK_ORDER])
    xp = np.concatenate([x_np, np.ones((NPAD, 1), np.float32)], axis=1)
    xTF = np.ascontiguousarray(xp[perm_nodes].T).astype(bfnp)   # [10, NPAD]
    nrmf = (TSCALE * (deg > 0) / np.sqrt(np.maximum(deg, 1.0))).astype(np.float32)
    nrmxf = np.ascontiguousarray(nrmf[perm_nodes].reshape(NCHUNK, P).T)
    W_lhsT = Wf.reshape(L, 2, P, H).transpose(2, 0, 1, 3).reshape(P, L * 2 * H)
    # gamma/beta column-major: col 4l+k = gamma[l] half k, 4l+2+k = beta[l]
    gm = np.asarray(gamma, np.float32).reshape(L, 2, P)
    bt = np.asarray(beta, np.float32).reshape(L, 2, P)
    gbc = np.empty((P, 4 * L), np.float32)
    for l in range(L):
        for k in range(2):
            gbc[:, 4 * l + k] = gm[l, k]
            gbc[:, 4 * l + 2 + k] = bt[l, k]
    embf = np.asarray(emb, np.float32)
    emb0 = np.ascontiguousarray(embf[:, 0, :])
    emb1 = np.ascontiguousarray(embf[:, 1, :])
    w1 = np.asarray(mlp_W1, np.float32).reshape(2, P, P).transpose(1, 0, 2).reshape(P, 2 * P)
    w2 = np.asarray(mlp_W2, np.float32)
    w3 = np.asarray(mlp_W3, np.float32)
    b1 = np.asarray(mlp_b1, np.float32).reshape(P, 1)
    b2 = np.asarray(mlp_b2, np.float32).reshape(64, 1)
    b3 = np.asarray(mlp_b3, np.float32).reshape(1, 1)

    in_maps = []
    for c in range(NCORE):
        lo, hi = c * NPC, (c + 1) * NPC
        # fifth-major A tiles: tile s holds ALL 10 dst blocks' columns for
        # gather piece s (16 chunks each), block-major inside.
        Ab = np.stack([A_blocks[(c, nb)] for nb in range(BPC)], axis=1)
        Ac = Ab.reshape(P, BPC, NSEG, CPS * P).transpose(0, 2, 1, 3)
        Ac = np.ascontiguousarray(Ac).reshape(P, BPC * NCHUNK * P)

        degc = deg[lo:hi].reshape(BPC, P).T
        maskc = (degc > 0).astype(np.float32)
        degc = np.maximum(degc, 1.0).astype(np.float32)

        pselc = psel_full[lo:hi].reshape(BPC, P, G)
        pselc = np.ascontiguousarray(pselc.transpose(1, 0, 2)).reshape(P, BPC * G)

        in_maps.append(dict(
            A=Ac, xT=np.ascontiguousarray(x_np[lo:hi].T).astype(bfnp),
            xTF=xTF, DW0p=DW0p.astype(bfnp), nrmxf=nrmxf,
            deg=degc, mask=maskc, psel=pselc.astype(bfnp),
            W=W_lhsT.astype(bfnp), gbc=gbc, emb0=emb0, emb1=emb1,
            w1=w1, w2=w2, w3=w3, b1=b1, b2=b2, b3=b3, rcnt=rcnt,
        ))
    return in_maps


# --------------------------------------------------------------------------
# device program
# --------------------------------------------------------------------------

def _build():
    nc = bacc.Bacc(None, target_bir_lowering=False)

    d_A = nc.dram_tensor("A", [P, BPC * NCHUNK * P], f8, kind="ExternalInput")
    d_xT = nc.dram_tensor("xT", [NF, NPC], bf16, kind="ExternalInput")
    d_xTF = nc.dram_tensor("xTF", [NF + 1, NPAD], bf16, kind="ExternalInput")
    d_DW0p = nc.dram_tensor("DW0p", [NF + 1, H], bf16, kind="ExternalInput")
    d_nrmxf = nc.dram_tensor("nrmxf", [P, NCHUNK], f32, kind="ExternalInput")
    d_deg = nc.dram_tensor("deg", [P, BPC], f32, kind="ExternalInput")
    d_mask = nc.dram_tensor("mask", [P, BPC], f32, kind="ExternalInput")
    d_psel = nc.dram_tensor("psel", [P, BPC * G], bf16, kind="ExternalInput")
    d_W = nc.dram_tensor("W", [P, L * 2 * H], bf16, kind="ExternalInput")
    d_gbc = nc.dram_tensor("gbc", [P, 4 * L], f32, kind="ExternalInput")
    d_emb0 = nc.dram_tensor("emb0", [NF, H], f32, kind="ExternalInput")
    d_emb1 = nc.dram_tensor("emb1", [NF, H], f32, kind="ExternalInput")
    d_w1 = nc.dram_tensor("w1", [P, 2 * P], f32, kind="ExternalInput")
    d_w2 = nc.dram_tensor("w2", [P, 64], f32, kind="ExternalInput")
    d_w3 = nc.dram_tensor("w3", [64, 1], f32, kind="ExternalInput")
    d_b1 = nc.dram_tensor("b1", [P, 1], f32, kind="ExternalInput")
    d_b2 = nc.dram_tensor("b2", [64, 1], f32, kind="ExternalInput")
    d_b3 = nc.dram_tensor("b3", [1, 1], f32, kind="ExternalInput")
    d_rcnt = nc.dram_tensor("rcnt", [1, G], f32, kind="ExternalInput")
    d_out = nc.dram_tensor("out", [1, G], f32, kind="ExternalOutput")

    rg = [list(range(NCORE))]
    SW = BPS * H         # gather-piece payload width per partition (512 cols)

    with tile.TileContext(nc) as tc, ExitStack() as ctx:
        pers = ctx.enter_context(tc.tile_pool(name="pers", bufs=1))
        psA = ctx.enter_context(tc.tile_pool(name="psA", bufs=4, space="PSUM"))
        psB = ctx.enter_context(tc.tile_pool(name="psB", bufs=2, space="PSUM"))
        work = ctx.enter_context(tc.tile_pool(name="work", bufs=2))
        stream = ctx.enter_context(tc.tile_pool(name="stream", bufs=2))
        dram = ctx.enter_context(tc.tile_pool(name="dram", bufs=2, space="DRAM"))

        # ---- persistent SBUF state -------------------------------------
        deg_sb = pers.tile([P, BPC], f32, tag="deg")
        mask_sb = pers.tile([P, BPC], f32, tag="mask")
        psel_sb = pers.tile([P, BPC * G], bf16, tag="psel")
        W_sb = pers.tile([P, L * 2 * H], bf16, tag="W")
        gbc_sb = pers.tile([P, 4 * L], f32, tag="gbc")
        emb0_sb = pers.tile([NF, H], f32, tag="emb0")
        emb1_sb = pers.tile([NF, H], f32, tag="emb1")
        w1_sb = pers.tile([P, 2 * P], f32, tag="w1")
        w2_sb = pers.tile([P, 64], f32, tag="w2")
        w3_sb = pers.tile([64, 1], f32, tag="w3")
        b1_sb = pers.tile([P, 1], f32, tag="b1")
        b2_sb = pers.tile([64, 1], f32, tag="b2")
        b3_sb = pers.tile([1, 1], f32, tag="b3")

        tab_sb = pers.tile([P, NCHUNK * H], f8, tag="tab")
        hTb_sb = pers.tile([P, BPC * 2 * P], bf16, tag="hTb")
        hws_sb = pers.tile([P, BPC * H], f8, tag="hws")
        t_all = pers.tile([P, BPC * H], f32, tag="t_all")
        tT_sb = pers.tile([P, BPC * 2 * P], f32, tag="tT")
        nrm_sb = pers.tile([P, BPC], f32, tag="nrm")
        nrm32_sb = pers.tile([P, BPC], f32, tag="nrm32")
        nrm32x_sb = pers.tile([P, BPC], f32, tag="nrm32x")
        DW0p_sb = pers.tile([NF + 1, H], bf16, tag="DW0p")
        nrmxf_sb = pers.tile([P, NCHUNK], f32, tag="nrmxf")
        acc_sq = pers.tile([P, H], f32, tag="acc_sq")
        D_sb = pers.tile([NF, H], f32, tag="D")
        base_col = pers.tile([P, 2], f32, tag="base_col")
        bncol = pers.tile([P, 4], f32, tag="bncol")
        g_acc = pers.tile([P, 2 * G], f32, tag="g_acc")
        g_acc2 = pers.tile([P, 2 * G], f32, tag="g_acc2")
        ident_bf = pers.tile([P, P], bf16, tag="ident")
        ident_f = pers.tile([P, P], f32, tag="identf")
        ones9 = pers.tile([NF, 1], f32, tag="ones9")
        ones1 = pers.tile([1, P], f32, tag="ones1")
        ones128 = pers.tile([P, 8], f32, tag="ones128")
        stv = pers.tile([1, 2 * H], f32, tag="stv")
        rcnt_sb = pers.tile([1, G], f32, tag="rcnt")
        # the adjacency is layer-invariant: fully resident (13.1MB),
        # loaded once, never re-streamed
        a_res = [pers.tile([P, BPC * CPS * P], f8, tag=f"Ares{s}",
                           name=f"Ares{s}")
                 for s in range(NSEG)]

        # ---- DRAM bounce buffers ---------------------------------------
        # AllGather pieces: ag_in[s][p, :] = hws rows for blocks {2s,2s+1}
        # (512B fp8 contiguous run per partition; ag_out row c*128+p holds
        # core c's piece-run for partition p). Collective outputs are Shared
        # scratchpad (single-writer: one output tile per collective).
        ag_ins = [dram.tile([P, PW[p] * H], f8, tag=f"ag_in{p}",
                            name=f"ag_in{p}")
                  for p in range(len(PW))]
        ag_outs = {
            l: [dram.tile([NCORE * P, PW[p] * H], f8, tag=f"ag_out{p}_{l}",
                          bufs=1, name=f"ag_out{p}_{l}", addr_space="Shared")
                for p in range(len(PW))]
            for l in range(1, L)
        }
        RREP = 8             # BN stats replication rows (payload 16KB)
        ar_in = dram.tile([RREP, 2 * H], f32, tag="ar_in")
        ar_outs = [dram.tile([NCORE * RREP, 2 * H], f32, tag=f"ar_out_{l}",
                             bufs=1, name=f"ar_out_{l}", addr_space="Shared")
                   for l in range(L)]
        pr_in = dram.tile([2 * P, G], f32, tag="pr_in")
        pr_outA = dram.tile([P, G], f32, tag="pr_outA", bufs=1,
                            addr_space="Shared")
        pr_outB = dram.tile([P, G], f32, tag="pr_outB", bufs=1,
                            addr_space="Shared")
        # warmups matched to the BN-stats AllGather and pool-AllReduce
        # shape classes (collective setup cost is paid per class)
        warm_in2 = dram.tile([P, G], f32, tag="warm_in2")
        warm_out2 = dram.tile([P, G], f32, tag="warm_out2", bufs=1,
                              addr_space="Shared")
        warm_in3 = dram.tile([RREP, 2 * H], f32, tag="warm_in3")
        warm_out3 = dram.tile([NCORE * RREP, 2 * H], f32, tag="warm_out3",
                              bufs=1, addr_space="Shared")


        # warmup collective FIRST: absorbs the one-time comms boot +
        # core-arrival skew while the encoder runs. Collectives cannot read
        # IO tensors, so bounce a tiny staged input through Internal DRAM.
        nc.sync.dma_start(out=warm_in2[:], in_=d_w1[:, 0:G])
        nc.sync.dma_start(out=warm_in3[:],
                          in_=d_w1[0:2 * RREP, :].rearrange(
                              "(a b) w -> a (b w)", b=2))
        nc.gpsimd.collective_compute(
            "AllGather", OP.bypass, replica_groups=rg,
            ins=[warm_in3[:]], outs=[warm_out3[:]])
        nc.gpsimd.collective_compute(
            "AllReduce", OP.add, replica_groups=rg,
            ins=[warm_in2[:]], outs=[warm_out2[:]])

        # ---- input loads ------------------------------------------------
        # Small early-needed tensors go on the sync queue ahead of the
        # encoder's xT loads; bulk tensors ride the scalar/gpsimd queues so
        # they delay neither the encoder DMAs nor the first AllGather bounce.
        # only the encoder's inputs ride the sync queue ahead of the xT
        # stream; everything else (needed 25..150us later) goes to gpsimd
        for t, d in [(emb0_sb, d_emb0), (emb1_sb, d_emb1)]:
            nc.sync.dma_start(out=t[:], in_=d[:])
        # resident A: one fifth per DMA so layer-0 segment s never waits on
        # fifths it doesn't need yet
        for s in range(NSEG):
            nc.scalar.dma_start(
                out=a_res[s][:],
                in_=d_A[:, s * BPC * CPS * P:(s + 1) * BPC * CPS * P])
        for t, d in [(DW0p_sb, d_DW0p), (nrmxf_sb, d_nrmxf),
                     (deg_sb, d_deg), (mask_sb, d_mask), (W_sb, d_W),
                     (gbc_sb, d_gbc), (psel_sb, d_psel), (w1_sb, d_w1),
                     (w2_sb, d_w2), (w3_sb, d_w3), (b1_sb, d_b1),
                     (b2_sb, d_b2), (b3_sb, d_b3), (rcnt_sb, d_rcnt)]:
            nc.gpsimd.dma_start(out=t[:], in_=d[:])

        make_identity(nc, ident_bf[:])
        make_identity(nc, ident_f[:])
        nc.vector.memset(ones9[:], 1.0)
        nc.vector.memset(ones1[:], 1.0)
        nc.vector.memset(ones128[:], 1.0)
        nc.vector.memset(g_acc[:], 0.0)
        nc.vector.memset(g_acc2[:], 0.0)

        # encoder prep: D = emb1 - emb0 (bf16) ; base cols b_k = emb0_k^T @ 1
        nc.vector.tensor_tensor(out=D_sb[:], in0=emb1_sb[:], in1=emb0_sb[:], op=OP.subtract)
        D_bf = pers.tile([NF, H], bf16, tag="D_bf")
        nc.vector.tensor_copy(out=D_bf[:], in_=D_sb[:])
        for k in range(2):
            ps_b = psB.tile([P, 1], f32, tag="vec")
            nc.tensor.matmul(out=ps_b[:], lhsT=emb0_sb[:, k * P:(k + 1) * P],
                             rhs=ones9[:], start=True, stop=True)
            nc.vector.tensor_copy(out=base_col[:, k:k + 1], in_=ps_b[:])

        def hTb(nb, k):
            return hTb_sb[:, (nb * 2 + k) * P:(nb * 2 + k + 1) * P]

        def tT(nb, k):
            return tT_sb[:, (nb * 2 + k) * P:(nb * 2 + k + 1) * P]

        def emit_gemm(l, nb):
            """hws[nb] = (h @ W[l]) * nrm * 32, fp8. lhsT is h^T directly."""
            ps_g = psA.tile([P, H], f32, tag="mm")
            for k in range(2):
                nc.tensor.matmul(
                    out=ps_g[:], lhsT=hTb(nb, k),
                    rhs=W_sb[:, (l * 2 + k) * H:(l * 2 + k + 1) * H],
                    start=(k == 0), stop=(k == 1))
            nc.vector.tensor_scalar_mul(hws_sb[:, nb * H:(nb + 1) * H],
                                        ps_g[:], nrm32x_sb[:, nb:nb + 1])

        def emit_ag_piece(l, p):
            nc.sync.dma_start(
                out=ag_ins[p][:],
                in_=hws_sb[:, BOFF[p] * H:(BOFF[p] + PW[p]) * H])
            nc.gpsimd.collective_compute(
                "AllGather", OP.bypass, replica_groups=rg,
                ins=[ag_ins[p][:]], outs=[ag_outs[l][p][:]])

        # Aggregation runs in NSEG segments; segment s consumes gather piece
        # s for all 10 dst blocks, in chunk PAIRS via fp8 DoubleRow matmuls
        # (contraction 256/instruction at 2 fp8 rows/cycle).
        def emit_seg_chain(f, nb):
            ps_t = psA.tile([P, H], f32, tag="mm")
            for t in range(CPS // 2):
                lhsT3 = a_res[f][:, (nb * CPS + 2 * t) * P:
                                (nb * CPS + 2 * t + 2) * P].rearrange(
                    "p (j m) -> p j m", j=2)
                rhs3 = tab_sb[:, (f * CPS + 2 * t) * H:
                              (f * CPS + 2 * t + 2) * H].rearrange(
                    "p (j n) -> p j n", j=2)
                nc.tensor.matmul(
                    out=ps_t[:], lhsT=lhsT3, rhs=rhs3,
                    start=(t == 0), stop=(t == CPS // 2 - 1), perf_mode=DR)
            tsl = t_all[:, nb * H:(nb + 1) * H]
            if f == 0:
                nc.vector.tensor_copy(out=tsl, in_=ps_t[:])
            else:
                nc.vector.tensor_tensor(out=tsl, in0=tsl, in1=ps_t[:], op=OP.add)
            if f == NSEG - 1:
                # t = (nrm/32)*sum; sumsq accumulates on GpSimd (2 ops/block
                # keeps pace with the chains); the channel sums are batched
                # as PSUM matmuls after the last chain (TensorE is free).
                nc.vector.tensor_scalar_mul(tsl, tsl, nrm32_sb[:, nb:nb + 1])
                sq = work.tile([P, H], f32, tag="tmp3")
                # square on the (idle) Scalar engine, accumulate on GpSimd:
                # one op per engine per block keeps every queue ahead of the
                # chains, so the stats matmuls never wait on a drain
                nc.scalar.activation(out=sq[:], in_=tsl, func=FT.Square)
                nc.gpsimd.tensor_tensor(out=acc_sq[:], in0=acc_sq[:],
                                        in1=sq[:], op=OP.add)

        # encoder: h0^T = D^T x^T + base (directly transposed, bf16 carry)
        for nb in range(BPC):
            xT_t = stream.tile([NF, P], bf16, tag="xT_t", bufs=4)
            nc.sync.dma_start(out=xT_t[:], in_=d_xT[:, nb * P:(nb + 1) * P])
            for k in range(2):
                ps_h = psA.tile([P, H], f32, tag="mm")
                nc.tensor.matmul(out=ps_h[:, 0:P], lhsT=D_bf[:, k * P:(k + 1) * P],
                                 rhs=xT_t[:], start=True, stop=True)
                nc.vector.tensor_scalar_add(hTb(nb, k), ps_h[:, 0:P],
                                            base_col[:, k:k + 1])
            emit_gemm(0, nb)

        # nrm = rsqrt(deg) * mask ; the fp8 table is stored x32 (keeps hws
        # out of fp8e4m3 subnormals); the dst-side norm absorbs the 1/32.
        # Emitted after the encoder so the Sqrt's ACT-table load does not
        # contend with the layer-0 build casts on the Scalar engine.
        rdeg = work.tile([P, BPC], f32, tag="rdeg", bufs=1)
        nc.vector.reciprocal(out=rdeg[:], in_=deg_sb[:])
        nc.scalar.activation(out=rdeg[:], in_=rdeg[:], func=FT.Sqrt)
        nc.vector.tensor_tensor(out=nrm_sb[:], in0=rdeg[:], in1=mask_sb[:], op=OP.mult)
        nc.vector.tensor_scalar_mul(nrm32_sb[:], nrm_sb[:], 1.0 / TSCALE)
        nc.vector.tensor_scalar_mul(nrm32x_sb[:], nrm_sb[:], TSCALE)

        # ---- layers -----------------------------------------------------
        for l in range(L):
            if l > 0:
                # Table loads ride the gpsimd queue: each waits on its gather
                # piece, exactly the order the Comms engine completes them,
                # so nothing else ever queues behind a blocked trigger.
                # Piece 2 covers fifths 2+3 and is split per fifth so
                # segment 3 is never gated by fifth 4's bytes.
                for p, (fifth, off) in [(0, (0, 0)), (1, (1, 0)),
                                        (2, (2, 0)), (2, (3, 1)),
                                        (3, (4, 0))]:
                    nc.gpsimd.dma_start(
                        out=tab_sb[:, fifth * CPS * H:
                                   (fifth + 1) * CPS * H].rearrange(
                            "p (c w) -> p c w", c=NCORE),
                        in_=ag_outs[l][p][:, off * BPS * H:
                                          (off + 1) * BPS * H].rearrange(
                            "(c p) w -> p c w", p=P))
            nc.gpsimd.memset(acc_sq[:], 0.0)
            for seg in range(NSEG):
                for s in ([] if l > 0 else [seg]):
                    # build this fifth of the layer-0 table locally:
                    # tab[q] = (x_q @ DW0p) * nrm * 32, one matmul per chunk
                    for h2 in range(2):
                        xf_t = stream.tile([NF + 1, CPS * P // 2], bf16,
                                           tag="xtf", bufs=4)
                        base_q = s * CPS + h2 * (CPS // 2)
                        nc.sync.dma_start(
                            out=xf_t[:],
                            in_=d_xTF[:, base_q * P:(base_q + CPS // 2) * P])
                        for j in range(CPS // 2):
                            q = base_q + j
                            ps_q = psA.tile([P, H], f32, tag="mm")
                            nc.tensor.matmul(out=ps_q[:],
                                             lhsT=xf_t[:, j * P:(j + 1) * P],
                                             rhs=DW0p_sb[:],
                                             start=True, stop=True)
                            # scale+fp8-cast on the (idle) Scalar engine so
                            # the DVE queue stays clear for the seg adds
                            nc.scalar.activation(
                                out=tab_sb[:, q * H:(q + 1) * H], in_=ps_q[:],
                                func=FT.Copy, scale=nrmxf_sb[:, q:q + 1])
                for nb in range(BPC):
                    emit_seg_chain(seg, nb)

            # stats: channel sums via a PSUM-accumulated matmul batch (all t
            # blocks are final by now), sumsq from the GpSimd accumulator,
            # 32-row replicate, AllReduce
            # the ones-lhsT is 8 wide, so the stats matmuls produce all 8
            # replica rows directly: no separate replicate matmul/round-trip
            ps_stat = psB.tile([RREP, H], f32, tag="pst")
            for nb in range(BPC):
                nc.tensor.matmul(out=ps_stat[:], lhsT=ones128[:, 0:RREP],
                                 rhs=t_all[:, nb * H:(nb + 1) * H],
                                 start=(nb == 0), stop=(nb == BPC - 1))
            ps_q = psB.tile([RREP, H], f32, tag="pst")
            nc.tensor.matmul(out=ps_q[:], lhsT=ones128[:, 0:RREP], rhs=acc_sq[:],
                             start=True, stop=True)
            st_rep = work.tile([RREP, 2 * H], f32, tag="strep", bufs=1)
            nc.vector.tensor_copy(out=st_rep[:, 0:H], in_=ps_stat[:])
            nc.vector.tensor_copy(out=st_rep[:, H:2 * H], in_=ps_q[:])
            nc.sync.dma_start(out=ar_in[:], in_=st_rep[:])
            nc.gpsimd.collective_compute(
                "AllGather", OP.bypass, replica_groups=rg,
                ins=[ar_in[:]], outs=[ar_outs[l][:]])
            # one row per core is enough (the 8 replicas are identical):
            # 16KB instead of 128KB, and K=8 column-ize matmuls
            agst = work.tile([NCORE, 2 * H], f32, tag="agst", bufs=1)
            nc.sync.dma_start(
                out=agst[:],
                in_=ar_outs[l][:].rearrange("(c r) w -> c (r w)",
                                            r=RREP)[:, 0:2 * H])

            # transpose t into tT while the AllReduce is in flight (TensorE
            # is otherwise idle in this window)
            for nb in range(BPC):
                for k in range(2):
                    ps_t2 = psB.tile([P, P], f32, tag="pst")
                    nc.tensor.transpose(
                        out=ps_t2[:], in_=t_all[:, nb * H + k * P:nb * H + (k + 1) * P],
                        identity=ident_f[:])
                    nc.vector.tensor_copy(out=tT(nb, k), in_=ps_t2[:])

            # column-ize the returned stats (4 tiny matmuls [1,128]^T@[1,1]
            # -> [128,1]) and do ALL BN math on [128,2] columns: per-
            # partition ops are ~5x cheaper than [1,256] row ops, and a||c
            # come out directly in the per-partition form the apply needs.
            ps_col = psB.tile([P, 4], f32, tag="vec")
            for j in range(4):
                nc.tensor.matmul(out=ps_col[:, j:j + 1],
                                 lhsT=agst[:, j * P:(j + 1) * P],
                                 rhs=ones128[0:NCORE, 0:1],
                                 start=True, stop=True,
                                 skip_group_check=True)
            bnw = work.tile([P, 8], f32, tag="bnw")
            nc.vector.tensor_scalar_mul(bnw[:, 0:4], ps_col[:], 1.0 / N)
            mu = bnw[:, 0:2]
            var = bnw[:, 2:4]
            msq = bnw[:, 4:6]
            nc.vector.tensor_tensor(out=msq, in0=mu, in1=mu, op=OP.mult)
            nc.vector.tensor_tensor(out=var, in0=var, in1=msq, op=OP.subtract)
            nc.vector.tensor_scalar_add(var, var, BN_EPS)
            nc.vector.reciprocal_approx_fast(out=var, in_=var)
            nc.scalar.activation(out=var, in_=var, func=FT.Sqrt)  # istd
            nc.vector.tensor_tensor(out=bncol[:, 0:2], in0=var,
                                    in1=gbc_sb[:, 4 * l:4 * l + 2], op=OP.mult)
            nc.vector.tensor_tensor(out=msq, in0=mu, in1=bncol[:, 0:2], op=OP.mult)
            nc.vector.tensor_tensor(out=bncol[:, 2:4],
                                    in0=gbc_sb[:, 4 * l + 2:4 * l + 4],
                                    in1=msq, op=OP.subtract)

            # apply: h^T += relu(t^T * a + c) per (block, half); fused DVE
            # per-partition scalar op + Scalar-engine relu. Immediately GEMM
            # the updated block for the next layer; post the AllGather halves
            # as soon as each half's blocks are done.
            for nb in range(BPC):
                for k in range(2):
                    # fused BN apply: relu(t*a + c) in ONE Scalar-engine op
                    # (per-partition scale/bias APs)
                    r = work.tile([P, P], bf16, tag="tmp2")
                    nc.scalar.activation(out=r[:], in_=tT(nb, k), func=FT.Relu,
                                         scale=bncol[:, k:k + 1],
                                         bias=bncol[:, 2 + k:3 + k])
                    nc.vector.tensor_tensor(out=hTb(nb, k), in0=hTb(nb, k),
                                            in1=r[:], op=OP.add)
                if l < L - 1:
                    emit_gemm(l + 1, nb)
                    if nb == 1:
                        emit_ag_piece(l + 1, 0)
                    elif nb == 3:
                        emit_ag_piece(l + 1, 1)
                    elif nb == 7:
                        emit_ag_piece(l + 1, 2)
                    elif nb == 9:
                        emit_ag_piece(l + 1, 3)
                else:
                    # last layer: transpose back to node-major, pool matmuls,
                    # accumulate in SBUF (keeps PSUM banks free for chains)
                    hb_t = work.tile([P, H], bf16, tag="hb")
                    for k in range(2):
                        ps_tr = psB.tile([P, P], bf16, tag="pst")
                        nc.tensor.transpose(out=ps_tr[:], in_=hTb(nb, k),
                                            identity=ident_bf[:])
                        nc.vector.tensor_copy(out=hb_t[:, k * P:(k + 1) * P],
                                              in_=ps_tr[:])
                    pssl = psel_sb[:, nb * G:(nb + 1) * G]
                    ga = g_acc if nb < 3 else g_acc2
                    for k in range(2):
                        ps_p = psB.tile([P, G], f32, tag="vec")
                        nc.tensor.matmul(out=ps_p[:], lhsT=hb_t[:, k * P:(k + 1) * P],
                                         rhs=pssl, start=True, stop=True)
                        nc.vector.tensor_tensor(
                            out=ga[:, k * G:(k + 1) * G],
                            in0=ga[:, k * G:(k + 1) * G], in1=ps_p[:], op=OP.add)
                    if nb == 2:
                        # early pool fold + AllReduce overlaps the rest
                        # of the apply/pool work (3/7 split: the second
                        # collective's input is ready right as the first
                        # clears the serial Comms engine)
                        ps_za = psB.tile([P, G], f32, tag="vec")
                        nc.tensor.matmul(out=ps_za[:], lhsT=w1_sb[:, 0:P],
                                         rhs=g_acc[:, 0:G], start=True, stop=False)
                        nc.tensor.matmul(out=ps_za[:], lhsT=w1_sb[:, P:2 * P],
                                         rhs=g_acc[:, G:2 * G], start=False, stop=True)
                        za = work.tile([P, G], f32, tag="za", bufs=1)
                        nc.vector.tensor_copy(out=za[:], in_=ps_za[:])
                        nc.sync.dma_start(out=pr_in[0:P, :], in_=za[:])
                        nc.gpsimd.collective_compute(
                            "AllReduce", OP.add, replica_groups=rg,
                            ins=[pr_in[0:P, :]], outs=[pr_outA[:]])

        # ---- pooling readout --------------------------------------------
        # fold the MLP's first matmul BEFORE the AllReduce (linear in the
        # pool sums; rcnt is a per-graph column scale and commutes): halves
        # the collective payload to 128KB.
        ps_z = psB.tile([P, G], f32, tag="vec")
        nc.tensor.matmul(out=ps_z[:], lhsT=w1_sb[:, 0:P], rhs=g_acc2[:, 0:G],
                         start=True, stop=False)
        nc.tensor.matmul(out=ps_z[:], lhsT=w1_sb[:, P:2 * P], rhs=g_acc2[:, G:2 * G],
                         start=False, stop=True)
        zt = work.tile([P, G], f32, tag="g0", bufs=1)
        nc.vector.tensor_copy(out=zt[:], in_=ps_z[:])
        nc.sync.dma_start(out=pr_in[P:2 * P, :], in_=zt[:])
        nc.gpsimd.collective_compute(
            "AllReduce", OP.add, replica_groups=rg,
            ins=[pr_in[P:2 * P, :]], outs=[pr_outB[:]])
        zs = work.tile([P, G], f32, tag="g1", bufs=1)
        nc.sync.dma_start(out=zs[:], in_=pr_outA[:])
        zs2 = work.tile([P, G], f32, tag="g2", bufs=1)
        nc.sync.dma_start(out=zs2[:], in_=pr_outB[:])
        ps_r = psB.tile([P, G], f32, tag="vec")
        nc.tensor.matmul(out=ps_r[:], lhsT=ones1[:], rhs=rcnt_sb[:], start=True, stop=True)
        rc_rep = work.tile([P, G], f32, tag="rc_rep", bufs=1)
        nc.vector.tensor_copy(out=rc_rep[:], in_=ps_r[:])
        nc.vector.tensor_tensor(out=zs[:], in0=zs[:], in1=zs2[:], op=OP.add)
        nc.vector.tensor_tensor(out=zs[:], in0=zs[:], in1=rc_rep[:], op=OP.mult)
        y1 = work.tile([P, G], f32, tag="y1", bufs=1)
        nc.scalar.activation(out=y1[:], in_=zs[:], func=FT.Relu, bias=b1_sb[:, 0:1])
        ps2 = psB.tile([64, G], f32, tag="vec")
        nc.tensor.matmul(out=ps2[:], lhsT=w2_sb[:], rhs=y1[:], start=True, stop=True)
        y2 = work.tile([64, G], f32, tag="y2", bufs=1)
        nc.scalar.activation(out=y2[:], in_=ps2[:], func=FT.Relu, bias=b2_sb[:, 0:1])
        ps3 = psB.tile([1, G], f32, tag="vec")
        nc.tensor.matmul(out=ps3[:], lhsT=w3_sb[:], rhs=y2[:], start=True, stop=True)
        y3 = work.tile([1, G], f32, tag="y3", bufs=1)
        nc.vector.tensor_scalar_add(y3[:], ps3[:], b3_sb[0:1, 0:1])
        nc.sync.dma_start(out=d_out[:], in_=y3[:])

    nc.compile()
    return nc


# --------------------------------------------------------------------------
# entry point
# --------------------------------------------------------------------------

def kernel(x, edge_index, batch_ids, emb, W, b, gamma, beta,
           mlp_W1, mlp_b1, mlp_W2, mlp_b2, mlp_W3, mlp_b3,
           _trace=False, _trace_kwargs=None):
    # NB: reference BN subtracts the per-channel mean, so the additive bias b
    # cancels exactly and is not needed by the device program.
    in_maps = _preprocess(x, edge_index, batch_ids, emb, W, gamma, beta,
                          mlp_W1, mlp_b1, mlp_W2, mlp_b2, mlp_W3, mlp_b3)
    if "nc" not in _compiled:
        _compiled["nc"] = _build()
    nc = _compiled["nc"]
    kw = {}
    if _trace:
        kw = dict(trace=True, **(_trace_kwargs or {}))
    res = run_bass_kernel_spmd(nc, in_maps, core_ids=list(range(NCORE)), **kw)
    out = np.asarray(res.results[0]["out"], np.float32).reshape(G, 1)
    kernel._last_results = res
    return out


# revision 120
# speedup vs baseline: 1.0455x; 1.0455x over previous
"""Trainium2 Bass kernel for HIVNet GCN message passing (8-core SPMD).

Final design (baseline 826us -> ~500us; lineage: v7 fp8 DoubleRow 638us,
v8 transposed-h 628us, v11 A-resident 554us):
  - Pad N=10000 nodes to 10240 = 80 chunks x 128; core c owns 10 dst-blocks.
  - Aggregation = dense one-hot adjacency on TensorE via fp8e4m3 DoubleRow
    matmuls (both operands fp8, contraction 256/instruction, 2x bf16 rate).
    A (edge multiplicities, exact in fp8) is layer-invariant and fully
    SBUF-RESIDENT (13.1MB, loaded once) - no per-layer restream, no SBUF
    write contention against the chains.
  - Per layer: hws = (h @ W[l])*nrm*32 fp8 on the owned shard (x32 keeps the
    table out of fp8 subnormals; the dst-side norm carries 1/32), AllGathered
    in 4 pieces {2,2,4,2} posted progressively during the BN-apply loop;
    aggregation runs in 5 segments that consume each piece as it lands
    (Shared-output collectives; tab loads ride the gpsimd queue so a
    blocked trigger never stalls anything else).
  - Layer 0 needs NO gather: every core builds the full layer-0 table
    locally from the (tiny) full feature matrix via
    h0@W0 = x@(D@W0) + base@W0, one 10x128x256 matmul per chunk.
  - h lives TRANSPOSED in bf16 (H on partitions): the GEMM consumes h^T
    directly as lhsT; BN apply is ONE fused Scalar-engine op
    relu(t^T*scale + bias) with per-partition a,c + a bf16 DVE residual add.
  - BN stats: channel sums via PSUM-accumulated ones-matmuls, square on
    the Scalar engine + accumulate on GpSimd (one op per engine per block
    keeps every queue ahead of the chains), then a Shared ALLGATHER whose 8
    replica rows come straight from the 8-wide ones-lhsT (cheaper than an AllReduce: one row per core is read
    back and the cross-core reduction folds for free into the 4 K=8
    column-ize matmuls); t^T transposes fill the collective window and all
    BN math runs on [128,2] columns.
  - Warmup AllGather+AllReduce at t=0 absorb the ~60us comms boot under the
    encoder; input loads are queue-routed by first-use time.
  - Readout: W1 is folded into the pool sums BEFORE the 128KB pool
    AllReduce (split in two, first half posted early), then rcnt/relu and
    the rest of the MLP run redundantly on every core.
"""

import os
import sys

sys.path.insert(0, "/opt/trn_rl_repo")

from contextlib import ExitStack

import numpy as np
import ml_dtypes

from concourse import bass, mybir, bacc, tile, library_config
from concourse.bass_utils import run_bass_kernel_spmd
from concourse.masks import make_identity

NCORE = 8
P = 128
H = 256
L = 4
NF = 9
G = 256
N = 10000
BPC = 10                # dst blocks per core
NPC = BPC * P           # 1280 nodes per core
NPAD = NCORE * NPC      # 10240
NCHUNK = NPAD // P      # 80 src chunks
HB = BPC // 2           # blocks per AllGather half
BN_EPS = 1e-5
TSCALE = 32.0           # fp8 table scale

f32 = mybir.dt.float32
bf16 = mybir.dt.bfloat16
f8 = mybir.dt.float8e4
bfnp = ml_dtypes.bfloat16

FT = mybir.ActivationFunctionType
OP = mybir.AluOpType
DR = mybir.MatmulPerfMode.DoubleRow

_compiled = {}

NSEG = 5                # aggregation segments per layer (2 blocks each)
BPS = BPC // NSEG       # blocks per segment
CPS = NCORE * BPS       # chunks per segment (16)
# AllGather piece geometry (layers 1..3): small leading pieces so segment 0
# can start early, the 4-block piece in the middle (posted as soon as block
# 7 is ready) so the trailing fifths land with slack.
PW = [2, 2, 4, 2]       # blocks per gather piece
BOFF = [0, 2, 4, 8]     # first block of each piece

# chunk consumption order: fifth-major (blocks {2s,2s+1} of every core form
# table segment s); within a segment, core-major ascending = the gathered
# tab layout.
CHUNK_ORDER = [g for s in range(NSEG) for g in range(NCHUNK)
               if g % BPC in (2 * s, 2 * s + 1)]


# --------------------------------------------------------------------------
# host-side structural preprocessing
# --------------------------------------------------------------------------

def _preprocess(x, edge_index, batch_ids, emb, W, gamma, beta,
                mlp_W1, mlp_b1, mlp_W2, mlp_b2, mlp_W3, mlp_b3):
    src = np.asarray(edge_index[0], np.int64)
    dst = np.asarray(edge_index[1], np.int64)
    # self loops for every real node (weight nrm[d]^2 folds in)
    src_all = np.concatenate([src, np.arange(N, dtype=np.int64)])
    dst_all = np.concatenate([dst, np.arange(N, dtype=np.int64)])
    order = np.argsort(dst_all, kind="stable")
    s_sorted = src_all[order]
    d_sorted = dst_all[order]

    deg = np.bincount(dst_all, minlength=NPAD).astype(np.float64)  # incl self

    nblk = NCORE * BPC
    starts = np.searchsorted(d_sorted, np.arange(nblk) * P)
    ends = np.searchsorted(d_sorted, (np.arange(nblk) + 1) * P)

    # dense adjacency per dst block, chunk-major in CHUNK_ORDER. A is
    # layer-invariant and lives fully resident in SBUF on-device (13.1MB),
    # so own chunks stay in-place (no local/streamed split).
    A_blocks = {}
    for g in range(nblk):
        c, nb = divmod(g, BPC)
        e_s = s_sorted[starts[g]:ends[g]]
        e_d = d_sorted[starts[g]:ends[g]] - g * P
        A = np.zeros((NPAD, P), np.float32)
        np.add.at(A, (e_s, e_d), 1.0)
        A = A.reshape(NCHUNK, P, P)
        A = A[CHUNK_ORDER]                                # reorder chunks
        # fp8 e4m3: edge multiplicities (<= 3 incl. self loop) are exact,
        # and fp8 x fp8 DoubleRow matmul runs at 2x bf16 throughput.
        A_blocks[(c, nb)] = np.ascontiguousarray(
            A.transpose(1, 0, 2).reshape(P, NCHUNK * P)
        ).astype(ml_dtypes.float8_e4m3)

    # graph pool one-hot [node, graph] (bf16: values 0/1 exact)
    bids = np.asarray(batch_ids, np.int64)
    psel_full = np.zeros((NPAD, G), np.float32)
    psel_full[np.arange(N), bids] = 1.0
    cnt = np.bincount(bids, minlength=G).astype(np.float64)
    rcnt = (1.0 / np.maximum(cnt, 1.0)).astype(np.float32)[None, :]

    x_np = np.zeros((NPAD, NF), np.float32)
    x_np[:N] = np.asarray(x, np.float64)

    Wf = np.asarray(W, np.float32)
    embf0 = np.asarray(emb, np.float32)

    # layer-0 table is built LOCALLY on every core (no gather): the full
    # feature matrix is tiny, and h0@W0 = x@(D@W0) + base@W0 collapses the
    # encoder+GEMM into one 10x128x256 matmul per chunk. All in tab-position
    # (fifth-major CHUNK_ORDER) node order.
    Df = embf0[:, 1, :] - embf0[:, 0, :]            # [9, H]
    basef = embf0[:, 0, :].sum(axis=0)              # [H]
    DW0p = np.concatenate([Df @ Wf[0], (basef @ Wf[0])[None, :]], axis=0)
    perm_nodes = np.concatenate(
        [np.arange(g * P, (g + 1) * P) for g in CHUNK_ORDER])
    xp = np.concatenate([x_np, np.ones((NPAD, 1), np.float32)], axis=1)
    xTF = np.ascontiguousarray(xp[perm_nodes].T).astype(bfnp)   # [10, NPAD]
    nrmf = (TSCALE * (deg > 0) / np.sqrt(np.maximum(deg, 1.0))).astype(np.float32)
    nrmxf = np.ascontiguousarray(nrmf[perm_nodes].reshape(NCHUNK, P).T)
    W_lhsT = Wf.reshape(L, 2, P, H).transpose(2, 0, 1, 3).reshape(P, L * 2 * H)
    # gamma/beta column-major: col 4l+k = gamma[l] half k, 4l+2+k = beta[l]
    gm = np.asarray(gamma, np.float32).reshape(L, 2, P)
    bt = np.asarray(beta, np.float32).reshape(L, 2, P)
    gbc = np.empty((P, 4 * L), np.float32)
    for l in range(L):
        for k in range(2):
            gbc[:, 4 * l + k] = gm[l, k]
            gbc[:, 4 * l + 2 + k] = bt[l, k]
    embf = np.asarray(emb, np.float32)
    emb0 = np.ascontiguousarray(embf[:, 0, :])
    emb1 = np.ascontiguousarray(embf[:, 1, :])
    w1 = np.asarray(mlp_W1, np.float32).reshape(2, P, P).transpose(1, 0, 2).reshape(P, 2 * P)
    w2 = np.asarray(mlp_W2, np.float32)
    w3 = np.asarray(mlp_W3, np.float32)
    b1 = np.asarray(mlp_b1, np.float32).reshape(P, 1)
    b2 = np.asarray(mlp_b2, np.float32).reshape(64, 1)
    b3 = np.asarray(mlp_b3, np.float32).reshape(1, 1)

    in_maps = []
    for c in range(NCORE):
        lo, hi = c * NPC, (c + 1) * NPC
        # fifth-major A tiles: tile s holds ALL 10 dst blocks' columns for
        # gather piece s (16 chunks each), block-major inside.
        Ab = np.stack([A_blocks[(c, nb)] for nb in range(BPC)], axis=1)
        Ac = Ab.reshape(P, BPC, NSEG, CPS * P).transpose(0, 2, 1, 3)
        Ac = np.ascontiguousarray(Ac).reshape(P, BPC * NCHUNK * P)

        degc = deg[lo:hi].reshape(BPC, P).T
        maskc = (degc > 0).astype(np.float32)
        degc = np.maximum(degc, 1.0).astype(np.float32)

        pselc = psel_full[lo:hi].reshape(BPC, P, G)
        pselc = np.ascontiguousarray(pselc.transpose(1, 0, 2)).reshape(P, BPC * G)

        in_maps.append(dict(
            A=Ac, xT=np.ascontiguousarray(x_np[lo:hi].T).astype(bfnp),
            xTF=xTF, DW0p=DW0p.astype(bfnp), nrmxf=nrmxf,
            deg=degc, mask=maskc, psel=pselc.astype(bfnp),
            W=W_lhsT.astype(bfnp), gbc=gbc, emb0=emb0, emb1=emb1,
            w1=w1, w2=w2, w3=w3, b1=b1, b2=b2, b3=b3, rcnt=rcnt,
        ))
    return in_maps


# --------------------------------------------------------------------------
# device program
# --------------------------------------------------------------------------

def _build():
    nc = bacc.Bacc(None, target_bir_lowering=False)

    d_A = nc.dram_tensor("A", [P, BPC * NCHUNK * P], f8, kind="ExternalInput")
    d_xT = nc.dram_tensor("xT", [NF, NPC], bf16, kind="ExternalInput")
    d_xTF = nc.dram_tensor("xTF", [NF + 1, NPAD], bf16, kind="ExternalInput")
    d_DW0p = nc.dram_tensor("DW0p", [NF + 1, H], bf16, kind="ExternalInput")
    d_nrmxf = nc.dram_tensor("nrmxf", [P, NCHUNK], f32, kind="ExternalInput")
    d_deg = nc.dram_tensor("deg", [P, BPC], f32, kind="ExternalInput")
    d_mask = nc.dram_tensor("mask", [P, BPC], f32, kind="ExternalInput")
    d_psel = nc.dram_tensor("psel", [P, BPC * G], bf16, kind="ExternalInput")
    d_W = nc.dram_tensor("W", [P, L * 2 * H], bf16, kind="ExternalInput")
    d_gbc = nc.dram_tensor("gbc", [P, 4 * L], f32, kind="ExternalInput")
    d_emb0 = nc.dram_tensor("emb0", [NF, H], f32, kind="ExternalInput")
    d_emb1 = nc.dram_tensor("emb1", [NF, H], f32, kind="ExternalInput")
    d_w1 = nc.dram_tensor("w1", [P, 2 * P], f32, kind="ExternalInput")
    d_w2 = nc.dram_tensor("w2", [P, 64], f32, kind="ExternalInput")
    d_w3 = nc.dram_tensor("w3", [64, 1], f32, kind="ExternalInput")
    d_b1 = nc.dram_tensor("b1", [P, 1], f32, kind="ExternalInput")
    d_b2 = nc.dram_tensor("b2", [64, 1], f32, kind="ExternalInput")
    d_b3 = nc.dram_tensor("b3", [1, 1], f32, kind="ExternalInput")
    d_rcnt = nc.dram_tensor("rcnt", [1, G], f32, kind="ExternalInput")
    d_out = nc.dram_tensor("out", [1, G], f32, kind="ExternalOutput")

    rg = [list(range(NCORE))]
    SW = BPS * H         # gather-piece payload width per partition (512 cols)

    with tile.TileContext(nc) as tc, ExitStack() as ctx:
        pers = ctx.enter_context(tc.tile_pool(name="pers", bufs=1))
        psA = ctx.enter_context(tc.tile_pool(name="psA", bufs=4, space="PSUM"))
        psB = ctx.enter_context(tc.tile_pool(name="psB", bufs=2, space="PSUM"))
        work = ctx.enter_context(tc.tile_pool(name="work", bufs=2))
        stream = ctx.enter_context(tc.tile_pool(name="stream", bufs=2))
        dram = ctx.enter_context(tc.tile_pool(name="dram", bufs=2, space="DRAM"))

        # ---- persistent SBUF state -------------------------------------
        deg_sb = pers.tile([P, BPC], f32, tag="deg")
        mask_sb = pers.tile([P, BPC], f32, tag="mask")
        psel_sb = pers.tile([P, BPC * G], bf16, tag="psel")
        W_sb = pers.tile([P, L * 2 * H], bf16, tag="W")
        gbc_sb = pers.tile([P, 4 * L], f32, tag="gbc")
        emb0_sb = pers.tile([NF, H], f32, tag="emb0")
        emb1_sb = pers.tile([NF, H], f32, tag="emb1")
        w1_sb = pers.tile([P, 2 * P], f32, tag="w1")
        w2_sb = pers.tile([P, 64], f32, tag="w2")
        w3_sb = pers.tile([64, 1], f32, tag="w3")
        b1_sb = pers.tile([P, 1], f32, tag="b1")
        b2_sb = pers.tile([64, 1], f32, tag="b2")
        b3_sb = pers.tile([1, 1], f32, tag="b3")

        tab_sb = pers.tile([P, NCHUNK * H], f8, tag="tab")
        hTb_sb = pers.tile([P, BPC * 2 * P], bf16, tag="hTb")
        hws_sb = pers.tile([P, BPC * H], f8, tag="hws")
        t_all = pers.tile([P, BPC * H], f32, tag="t_all")
        tT_sb = pers.tile([P, BPC * 2 * P], f32, tag="tT")
        nrm_sb = pers.tile([P, BPC], f32, tag="nrm")
        nrm32_sb = pers.tile([P, BPC], f32, tag="nrm32")
        nrm32x_sb = pers.tile([P, BPC], f32, tag="nrm32x")
        DW0p_sb = pers.tile([NF + 1, H], bf16, tag="DW0p")
        nrmxf_sb = pers.tile([P, NCHUNK], f32, tag="nrmxf")
        acc_sq = pers.tile([P, H], f32, tag="acc_sq")
        D_sb = pers.tile([NF, H], f32, tag="D")
        base_col = pers.tile([P, 2], f32, tag="base_col")
        bncol = pers.tile([P, 4], f32, tag="bncol")
        g_acc = pers.tile([P, 2 * G], f32, tag="g_acc")
        g_acc2 = pers.tile([P, 2 * G], f32, tag="g_acc2")
        ident_bf = pers.tile([P, P], bf16, tag="ident")
        ident_f = pers.tile([P, P], f32, tag="identf")
        ones9 = pers.tile([NF, 1], f32, tag="ones9")
        ones1 = pers.tile([1, P], f32, tag="ones1")
        ones128 = pers.tile([P, 8], f32, tag="ones128")
        stv = pers.tile([1, 2 * H], f32, tag="stv")
        rcnt_sb = pers.tile([1, G], f32, tag="rcnt")
        # the adjacency is layer-invariant: fully resident (13.1MB),
        # loaded once, never re-streamed
        a_res = [pers.tile([P, BPC * CPS * P], f8, tag=f"Ares{s}",
                           name=f"Ares{s}")
                 for s in range(NSEG)]

        # ---- DRAM bounce buffers ---------------------------------------
        # AllGather pieces: ag_in[s][p, :] = hws rows for blocks {2s,2s+1}
        # (512B fp8 contiguous run per partition; ag_out row c*128+p holds
        # core c's piece-run for partition p). Collective outputs are Shared
        # scratchpad (single-writer: one output tile per collective).
        ag_ins = [dram.tile([P, PW[p] * H], f8, tag=f"ag_in{p}",
                            name=f"ag_in{p}")
                  for p in range(len(PW))]
        ag_outs = {
            l: [dram.tile([NCORE * P, PW[p] * H], f8, tag=f"ag_out{p}_{l}",
                          bufs=1, name=f"ag_out{p}_{l}", addr_space="Shared")
                for p in range(len(PW))]
            for l in range(1, L)
        }
        RREP = 8             # BN stats replication rows (payload 16KB)
        ar_in = dram.tile([RREP, 2 * H], f32, tag="ar_in")
        ar_outs = [dram.tile([NCORE * RREP, 2 * H], f32, tag=f"ar_out_{l}",
                             bufs=1, name=f"ar_out_{l}", addr_space="Shared")
                   for l in range(L)]
        pr_in = dram.tile([2 * P, G], f32, tag="pr_in")
        pr_outA = dram.tile([P, G], f32, tag="pr_outA", bufs=1,
                            addr_space="Shared")
        pr_outB = dram.tile([P, G], f32, tag="pr_outB", bufs=1,
                            addr_space="Shared")
        # warmups matched to the BN-stats AllGather and pool-AllReduce
        # shape classes (collective setup cost is paid per class)
        warm_in2 = dram.tile([P, G], f32, tag="warm_in2")
        warm_out2 = dram.tile([P, G], f32, tag="warm_out2", bufs=1,
                              addr_space="Shared")
        warm_in3 = dram.tile([RREP, 2 * H], f32, tag="warm_in3")
        warm_out3 = dram.tile([NCORE * RREP, 2 * H], f32, tag="warm_out3",
                              bufs=1, addr_space="Shared")


        # warmup collective FIRST: absorbs the one-time comms boot +
        # core-arrival skew while the encoder runs. Collectives cannot read
        # IO tensors, so bounce a tiny staged input through Internal DRAM.
        nc.sync.dma_start(out=warm_in2[:], in_=d_w1[:, 0:G])
        nc.sync.dma_start(out=warm_in3[:],
                          in_=d_w1[0:2 * RREP, :].rearrange(
                              "(a b) w -> a (b w)", b=2))
        nc.gpsimd.collective_compute(
            "AllGather", OP.bypass, replica_groups=rg,
            ins=[warm_in3[:]], outs=[warm_out3[:]])
        nc.gpsimd.collective_compute(
            "AllReduce", OP.add, replica_groups=rg,
            ins=[warm_in2[:]], outs=[warm_out2[:]])

        # ---- input loads ------------------------------------------------
        # Small early-needed tensors go on the sync queue ahead of the
        # encoder's xT loads; bulk tensors ride the scalar/gpsimd queues so
        # they delay neither the encoder DMAs nor the first AllGather bounce.
        # only the encoder's inputs ride the sync queue ahead of the xT
        # stream; everything else (needed 25..150us later) goes to gpsimd
        for t, d in [(emb0_sb, d_emb0), (emb1_sb, d_emb1)]:
            nc.sync.dma_start(out=t[:], in_=d[:])
        # resident A: one fifth per DMA so layer-0 segment s never waits on
        # fifths it doesn't need yet
        for s in range(NSEG):
            nc.scalar.dma_start(
                out=a_res[s][:],
                in_=d_A[:, s * BPC * CPS * P:(s + 1) * BPC * CPS * P])
        for t, d in [(DW0p_sb, d_DW0p), (nrmxf_sb, d_nrmxf),
                     (deg_sb, d_deg), (mask_sb, d_mask), (W_sb, d_W),
                     (gbc_sb, d_gbc), (psel_sb, d_psel), (w1_sb, d_w1),
                     (w2_sb, d_w2), (w3_sb, d_w3), (b1_sb, d_b1),
                     (b2_sb, d_b2), (b3_sb, d_b3), (rcnt_sb, d_rcnt)]:
            nc.gpsimd.dma_start(out=t[:], in_=d[:])

        make_identity(nc, ident_bf[:])
        make_identity(nc, ident_f[:])
        nc.vector.memset(ones9[:], 1.0)
        nc.vector.memset(ones1[:], 1.0)
        nc.vector.memset(ones128[:], 1.0)
        nc.vector.memset(g_acc[:], 0.0)
        nc.vector.memset(g_acc2[:], 0.0)

        # encoder prep: D = emb1 - emb0 (bf16) ; base cols b_k = emb0_k^T @ 1
        nc.vector.tensor_tensor(out=D_sb[:], in0=emb1_sb[:], in1=emb0_sb[:], op=OP.subtract)
        D_bf = pers.tile([NF, H], bf16, tag="D_bf")
        nc.vector.tensor_copy(out=D_bf[:], in_=D_sb[:])
        for k in range(2):
            ps_b = psB.tile([P, 1], f32, tag="vec")
            nc.tensor.matmul(out=ps_b[:], lhsT=emb0_sb[:, k * P:(k + 1) * P],
                             rhs=ones9[:], start=True, stop=True)
            nc.vector.tensor_copy(out=base_col[:, k:k + 1], in_=ps_b[:])

        def hTb(nb, k):
            return hTb_sb[:, (nb * 2 + k) * P:(nb * 2 + k + 1) * P]

        def tT(nb, k):
            return tT_sb[:, (nb * 2 + k) * P:(nb * 2 + k + 1) * P]

        def emit_gemm(l, nb):
            """hws[nb] = (h @ W[l]) * nrm * 32, fp8. lhsT is h^T directly."""
            ps_g = psA.tile([P, H], f32, tag="mm")
            for k in range(2):
                nc.tensor.matmul(
                    out=ps_g[:], lhsT=hTb(nb, k),
                    rhs=W_sb[:, (l * 2 + k) * H:(l * 2 + k + 1) * H],
                    start=(k == 0), stop=(k == 1))
            nc.vector.tensor_scalar_mul(hws_sb[:, nb * H:(nb + 1) * H],
                                        ps_g[:], nrm32x_sb[:, nb:nb + 1])

        def emit_ag_piece(l, p):
            nc.sync.dma_start(
                out=ag_ins[p][:],
                in_=hws_sb[:, BOFF[p] * H:(BOFF[p] + PW[p]) * H])
            nc.gpsimd.collective_compute(
                "AllGather", OP.bypass, replica_groups=rg,
                ins=[ag_ins[p][:]], outs=[ag_outs[l][p][:]])

        # Aggregation runs in NSEG segments; segment s consumes gather piece
        # s for all 10 dst blocks, in chunk PAIRS via fp8 DoubleRow matmuls
        # (contraction 256/instruction at 2 fp8 rows/cycle).
        def emit_seg_chain(f, nb):
            ps_t = psA.tile([P, H], f32, tag="mm")
            for t in range(CPS // 2):
                lhsT3 = a_res[f][:, (nb * CPS + 2 * t) * P:
                                (nb * CPS + 2 * t + 2) * P].rearrange(
                    "p (j m) -> p j m", j=2)
                rhs3 = tab_sb[:, (f * CPS + 2 * t) * H:
                              (f * CPS + 2 * t + 2) * H].rearrange(
                    "p (j n) -> p j n", j=2)
                nc.tensor.matmul(
                    out=ps_t[:], lhsT=lhsT3, rhs=rhs3,
                    start=(t == 0), stop=(t == CPS // 2 - 1), perf_mode=DR)
            tsl = t_all[:, nb * H:(nb + 1) * H]
            if f == 0:
                nc.vector.tensor_copy(out=tsl, in_=ps_t[:])
            else:
                nc.vector.tensor_tensor(out=tsl, in0=tsl, in1=ps_t[:], op=OP.add)
            if f == NSEG - 1:
                # t = (nrm/32)*sum; sumsq accumulates on GpSimd (2 ops/block
                # keeps pace with the chains); the channel sums are batched
                # as PSUM matmuls after the last chain (TensorE is free).
                nc.vector.tensor_scalar_mul(tsl, tsl, nrm32_sb[:, nb:nb + 1])
                sq = work.tile([P, H], f32, tag="tmp3")
                # square on the (idle) Scalar engine, accumulate on GpSimd:
                # one op per engine per block keeps every queue ahead of the
                # chains, so the stats matmuls never wait on a drain
                nc.scalar.activation(out=sq[:], in_=tsl, func=FT.Square)
                nc.gpsimd.tensor_tensor(out=acc_sq[:], in0=acc_sq[:],
                                        in1=sq[:], op=OP.add)

        # encoder: h0^T = D^T x^T + base (directly transposed, bf16 carry)
        for nb in range(BPC):
            xT_t = stream.tile([NF, P], bf16, tag="xT_t", bufs=4)
            nc.sync.dma_start(out=xT_t[:], in_=d_xT[:, nb * P:(nb + 1) * P])
            for k in range(2):
                ps_h = psA.tile([P, H], f32, tag="mm")
                nc.tensor.matmul(out=ps_h[:, 0:P], lhsT=D_bf[:, k * P:(k + 1) * P],
                                 rhs=xT_t[:], start=True, stop=True)
                nc.vector.tensor_scalar_add(hTb(nb, k), ps_h[:, 0:P],
                                            base_col[:, k:k + 1])
            emit_gemm(0, nb)

        # nrm = rsqrt(deg) * mask ; the fp8 table is stored x32 (keeps hws
        # out of fp8e4m3 subnormals); the dst-side norm absorbs the 1/32.
        # Emitted after the encoder so the Sqrt's ACT-table load does not
        # contend with the layer-0 build casts on the Scalar engine.
        rdeg = work.tile([P, BPC], f32, tag="rdeg", bufs=1)
        nc.vector.reciprocal(out=rdeg[:], in_=deg_sb[:])
        nc.scalar.activation(out=rdeg[:], in_=rdeg[:], func=FT.Sqrt)
        nc.vector.tensor_tensor(out=nrm_sb[:], in0=rdeg[:], in1=mask_sb[:], op=OP.mult)
        nc.vector.tensor_scalar_mul(nrm32_sb[:], nrm_sb[:], 1.0 / TSCALE)
        nc.vector.tensor_scalar_mul(nrm32x_sb[:], nrm_sb[:], TSCALE)

        # ---- layers -----------------------------------------------------
        for l in range(L):
            if l > 0:
                # Table loads ride the gpsimd queue: each waits on its gather
                # piece, exactly the order the Comms engine completes them,
                # so nothing else ever queues behind a blocked trigger.
                # Piece 2 covers fifths 2+3 and is split per fifth so
                # segment 3 is never gated by fifth 4's bytes.
                for p, (fifth, off) in [(0, (0, 0)), (1, (1, 0)),
                                        (2, (2, 0)), (2, (3, 1)),
                                        (3, (4, 0))]:
                    nc.gpsimd.dma_start(
                        out=tab_sb[:, fifth * CPS * H:
                                   (fifth + 1) * CPS * H].rearrange(
                            "p (c w) -> p c w", c=NCORE),
                        in_=ag_outs[l][p][:, off * BPS * H:
                                          (off + 1) * BPS * H].rearrange(
                            "(c p) w -> p c w", p=P))
            nc.gpsimd.memset(acc_sq[:], 0.0)
            for seg in range(NSEG):
                for s in ([] if l > 0 else [seg]):
                    # build this fifth of the layer-0 table locally:
                    # tab[q] = (x_q @ DW0p) * nrm * 32, one matmul per chunk
                    for h2 in range(2):
                        xf_t = stream.tile([NF + 1, CPS * P // 2], bf16,
                                           tag="xtf", bufs=4)
                        base_q = s * CPS + h2 * (CPS // 2)
                        nc.sync.dma_start(
                            out=xf_t[:],
                            in_=d_xTF[:, base_q * P:(base_q + CPS // 2) * P])
                        for j in range(CPS // 2):
                            q = base_q + j
                            ps_q = psA.tile([P, H], f32, tag="mm")
                            nc.tensor.matmul(out=ps_q[:],
                                             lhsT=xf_t[:, j * P:(j + 1) * P],
                                             rhs=DW0p_sb[:],
                                             start=True, stop=True)
                            # scale+fp8-cast on the (idle) Scalar engine so
                            # the DVE queue stays clear for the seg adds
                            nc.scalar.activation(
                                out=tab_sb[:, q * H:(q + 1) * H], in_=ps_q[:],
                                func=FT.Copy, scale=nrmxf_sb[:, q:q + 1])
                for nb in range(BPC):
                    emit_seg_chain(seg, nb)

            # stats: channel sums via a PSUM-accumulated matmul batch (all t
            # blocks are final by now), sumsq from the GpSimd accumulator,
            # 32-row replicate, AllReduce
            # the ones-lhsT is 8 wide, so the stats matmuls produce all 8
            # replica rows directly: no separate replicate matmul/round-trip
            ps_stat = psB.tile([RREP, H], f32, tag="pst")
            for nb in range(BPC):
                nc.tensor.matmul(out=ps_stat[:], lhsT=ones128[:, 0:RREP],
                                 rhs=t_all[:, nb * H:(nb + 1) * H],
                                 start=(nb == 0), stop=(nb == BPC - 1))
            ps_q = psB.tile([RREP, H], f32, tag="pst")
            nc.tensor.matmul(out=ps_q[:], lhsT=ones128[:, 0:RREP], rhs=acc_sq[:],
                             start=True, stop=True)
            st_rep = work.tile([RREP, 2 * H], f32, tag="strep", bufs=1)
            nc.vector.tensor_copy(out=st_rep[:, 0:H], in_=ps_stat[:])
            nc.vector.tensor_copy(out=st_rep[:, H:2 * H], in_=ps_q[:])
            nc.sync.dma_start(out=ar_in[:], in_=st_rep[:])
            nc.gpsimd.collective_compute(
                "AllGather", OP.bypass, replica_groups=rg,
                ins=[ar_in[:]], outs=[ar_outs[l][:]])
            # one row per core is enough (the 8 replicas are identical):
            # 16KB instead of 128KB, and K=8 column-ize matmuls
            agst = work.tile([NCORE, 2 * H], f32, tag="agst", bufs=1)
            nc.sync.dma_start(
                out=agst[:],
                in_=ar_outs[l][:].rearrange("(c r) w -> c (r w)",
                                            r=RREP)[:, 0:2 * H])

            # transpose t into tT while the AllReduce is in flight (TensorE
            # is otherwise idle in this window)
            for nb in range(BPC):
                for k in range(2):
                    ps_t2 = psB.tile([P, P], f32, tag="pst")
                    nc.tensor.transpose(
                        out=ps_t2[:], in_=t_all[:, nb * H + k * P:nb * H + (k + 1) * P],
                        identity=ident_f[:])
                    nc.vector.tensor_copy(out=tT(nb, k), in_=ps_t2[:])

            # column-ize the returned stats (4 tiny matmuls [1,128]^T@[1,1]
            # -> [128,1]) and do ALL BN math on [128,2] columns: per-
            # partition ops are ~5x cheaper than [1,256] row ops, and a||c
            # come out directly in the per-partition form the apply needs.
            ps_col = psB.tile([P, 4], f32, tag="vec")
            for j in range(4):
                nc.tensor.matmul(out=ps_col[:, j:j + 1],
                                 lhsT=agst[:, j * P:(j + 1) * P],
                                 rhs=ones128[0:NCORE, 0:1],
                                 start=True, stop=True,
                                 skip_group_check=True)
            bnw = work.tile([P, 8], f32, tag="bnw")
            nc.vector.tensor_scalar_mul(bnw[:, 0:4], ps_col[:], 1.0 / N)
            mu = bnw[:, 0:2]
            var = bnw[:, 2:4]
            msq = bnw[:, 4:6]
            nc.vector.tensor_tensor(out=msq, in0=mu, in1=mu, op=OP.mult)
            nc.vector.tensor_tensor(out=var, in0=var, in1=msq, op=OP.subtract)
            nc.vector.tensor_scalar_add(var, var, BN_EPS)
            nc.vector.reciprocal_approx_fast(out=var, in_=var)
            nc.scalar.activation(out=var, in_=var, func=FT.Sqrt)  # istd
            nc.vector.tensor_tensor(out=bncol[:, 0:2], in0=var,
                                    in1=gbc_sb[:, 4 * l:4 * l + 2], op=OP.mult)
            nc.vector.tensor_tensor(out=msq, in0=mu, in1=bncol[:, 0:2], op=OP.mult)
            nc.vector.tensor_tensor(out=bncol[:, 2:4],
                                    in0=gbc_sb[:, 4 * l + 2:4 * l + 4],
                                    in1=msq, op=OP.subtract)

            # apply: h^T += relu(t^T * a + c) per (block, half); fused DVE
            # per-partition scalar op + Scalar-engine relu. Immediately GEMM
            # the updated block for the next layer; post the AllGather halves
            # as soon as each half's blocks are done.
            for nb in range(BPC):
                for k in range(2):
                    # fused BN apply: relu(t*a + c) in ONE Scalar-engine op
                    # (per-partition scale/bias APs)
                    r = work.tile([P, P], bf16, tag="tmp2")
                    nc.scalar.activation(out=r[:], in_=tT(nb, k), func=FT.Relu,
                                         scale=bncol[:, k:k + 1],
                                         bias=bncol[:, 2 + k:3 + k])
                    nc.vector.tensor_tensor(out=hTb(nb, k), in0=hTb(nb, k),
                                            in1=r[:], op=OP.add)
                if l < L - 1:
                    emit_gemm(l + 1, nb)
                    if nb == 1:
                        emit_ag_piece(l + 1, 0)
                    elif nb == 3:
                        emit_ag_piece(l + 1, 1)
                    elif nb == 7:
                        emit_ag_piece(l + 1, 2)
                    elif nb == 9:
                        emit_ag_piece(l + 1, 3)
                else:
                    # last layer: transpose back to node-major, pool matmuls,
                    # accumulate in SBUF (keeps PSUM banks free for chains)
                    hb_t = work.tile([P, H], bf16, tag="hb")
                    for k in range(2):
                        ps_tr = psB.tile([P, P], bf16, tag="pst")
                        nc.tensor.transpose(out=ps_tr[:], in_=hTb(nb, k),
                                            identity=ident_bf[:])
                        nc.vector.tensor_copy(out=hb_t[:, k * P:(k + 1) * P],
                                              in_=ps_tr[:])
                    pssl = psel_sb[:, nb * G:(nb + 1) * G]
                    ga = g_acc if nb < 3 else g_acc2
                    for k in range(2):
                        ps_p = psB.tile([P, G], f32, tag="vec")
                        nc.tensor.matmul(out=ps_p[:], lhsT=hb_t[:, k * P:(k + 1) * P],
                                         rhs=pssl, start=True, stop=True)
                        nc.vector.tensor_tensor(
                            out=ga[:, k * G:(k + 1) * G],
                            in0=ga[:, k * G:(k + 1) * G], in1=ps_p[:], op=OP.add)
                    if nb == 2:
                        # early pool fold + AllReduce overlaps the rest
                        # of the apply/pool work (3/7 split: the second
                        # collective's input is ready right as the first
                        # clears the serial Comms engine)
                        ps_za = psB.tile([P, G], f32, tag="vec")
                        nc.tensor.matmul(out=ps_za[:], lhsT=w1_sb[:, 0:P],
                                         rhs=g_acc[:, 0:G], start=True, stop=False)
                        nc.tensor.matmul(out=ps_za[:], lhsT=w1_sb[:, P:2 * P],
                                         rhs=g_acc[:, G:2 * G], start=False, stop=True)
                        za = work.tile([P, G], f32, tag="za", bufs=1)
                        nc.vector.tensor_copy(out=za[:], in_=ps_za[:])
                        nc.sync.dma_start(out=pr_in[0:P, :], in_=za[:])
                        nc.gpsimd.collective_compute(
                            "AllReduce", OP.add, replica_groups=rg,
                            ins=[pr_in[0:P, :]], outs=[pr_outA[:]])

        # ---- pooling readout --------------------------------------------
        # fold the MLP's first matmul BEFORE the AllReduce (linear in the
        # pool sums; rcnt is a per-graph column scale and commutes): halves
        # the collective payload to 128KB.
        ps_z = psB.tile([P, G], f32, tag="vec")
        nc.tensor.matmul(out=ps_z[:], lhsT=w1_sb[:, 0:P], rhs=g_acc2[:, 0:G],
                         start=True, stop=False)
        nc.tensor.matmul(out=ps_z[:], lhsT=w1_sb[:, P:2 * P], rhs=g_acc2[:, G:2 * G],
                         start=False, stop=True)
        zt = work.tile([P, G], f32, tag="g0", bufs=1)
        nc.vector.tensor_copy(out=zt[:], in_=ps_z[:])
        nc.sync.dma_start(out=pr_in[P:2 * P, :], in_=zt[:])
        nc.gpsimd.collective_compute(
            "AllReduce", OP.add, replica_groups=rg,
            ins=[pr_in[P:2 * P, :]], outs=[pr_outB[:]])
        zs = work.tile([P, G], f32, tag="g1", bufs=1)
        nc.sync.dma_start(out=zs[:], in_=pr_outA[:])
        zs2 = work.tile([P, G], f32, tag="g2", bufs=1)
        nc.sync.dma_start(out=zs2[:], in_=pr_outB[:])
        ps_r = psB.tile([P, G], f32, tag="vec")
        nc.tensor.matmul(out=ps_r[:], lhsT=ones1[:], rhs=rcnt_sb[:], start=True, stop=True)
        rc_rep = work.tile([P, G], f32, tag="rc_rep", bufs=1)
        nc.vector.tensor_copy(out=rc_rep[:], in_=ps_r[:])
        nc.vector.tensor_tensor(out=zs[:], in0=zs[:], in1=zs2[:], op=OP.add)
        nc.vector.tensor_tensor(out=zs[:], in0=zs[:], in1=rc_rep[:], op=OP.mult)
        y1 = work.tile([P, G], f32, tag="y1", bufs=1)
        nc.scalar.activation(out=y1[:], in_=zs[:], func=FT.Relu, bias=b1_sb[:, 0:1])
        ps2 = psB.tile([64, G], f32, tag="vec")
        nc.tensor.matmul(out=ps2[:], lhsT=w2_sb[:], rhs=y1[:], start=True, stop=True)
        y2 = work.tile([64, G], f32, tag="y2", bufs=1)
        nc.scalar.activation(out=y2[:], in_=ps2[:], func=FT.Relu, bias=b2_sb[:, 0:1])
        ps3 = psB.tile([1, G], f32, tag="vec")
        nc.tensor.matmul(out=ps3[:], lhsT=w3_sb[:], rhs=y2[:], start=True, stop=True)
        y3 = work.tile([1, G], f32, tag="y3", bufs=1)
        nc.vector.tensor_scalar_add(y3[:], ps3[:], b3_sb[0:1, 0:1])
        nc.sync.dma_start(out=d_out[:], in_=y3[:])

    nc.compile()
    return nc


# --------------------------------------------------------------------------
# entry point
# --------------------------------------------------------------------------

def kernel(x, edge_index, batch_ids, emb, W, b, gamma, beta,
           mlp_W1, mlp_b1, mlp_W2, mlp_b2, mlp_W3, mlp_b3,
           _trace=False, _trace_kwargs=None):
    # NB: reference BN subtracts the per-channel mean, so the additive bias b
    # cancels exactly and is not needed by the device program.
    in_maps = _preprocess(x, edge_index, batch_ids, emb, W, gamma, beta,
                          mlp_W1, mlp_b1, mlp_W2, mlp_b2, mlp_W3, mlp_b3)
    if "nc" not in _compiled:
        _compiled["nc"] = _build()
    nc = _compiled["nc"]
    kw = {}
    if _trace:
        kw = dict(trace=True, **(_trace_kwargs or {}))
    res = run_bass_kernel_spmd(nc, in_maps, core_ids=list(range(NCORE)), **kw)
    out = np.asarray(res.results[0]["out"], np.float32).reshape(G, 1)
    kernel._last_results = res
    return out
